# revision 27
# baseline (speedup 1.0000x reference)
"""Trainium2 Bass kernel for nn_EnergyToRateConverter.

Computes Eyring rates  fwd = pref*exp(-(bar - G_from)/RT),
rev = reversible ? pref*exp(-(bar - G_to)/RT) : 0  for B=1M batch rows.

Strategy (pure data parallel over 8 cores, batch split 8 ways):
  * Host transposes inputs into one feature-major fp16 tensor
    X = [state.T; (barrier - C).T] of shape (80, B).  Subtracting the
    barrier mean C (~40) first puts barriers in the same fp16 binade as
    the state energies, so a single fp16 pass already hits ~1.3e-2
    worst-case relative error (gate is 2e-2) without a second
    residual-correction matmul pass.
  * One constant matmul W.T @ X per 512-column chunk fuses the
    per-transition gather AND the barrier subtraction:
        W[from_idx[j], j] = 1 (fwd cols) / W[to_idx[j], j] = 1 (rev)
        W[32+j, j] = -1  (subtract barrier j)
    Output rows are [48 fwd | n_rev rev] with no padding; rates for
    non-reversible transitions are never computed.
  * ScalarE evaluates out = exp(psum*inv_rt + (ln(pref) - C*inv_rt))
    straight from PSUM, writing bf16 (exponent range of f32, 2^-9
    rounding) — halving output DMA bytes vs f32.
  * DRAM tensors are tile-major: each 8192-column supertile is one
    contiguous 1.3/1.1 MB DRAM block (16KB per descriptor row), which
    keeps the SDMA engines in long sequential HBM bursts.
  * Output DMAs are issued as a 64-row chunk + 4-row remainder: the
    HWDGE spreads a DMA over the largest divisor of its descriptor
    count <= 16, so 64 rows ride all 16 SDMA engines (68 would use 4).
  * Traffic is split over three DMA queues (input -> SP HWDGE ring,
    output -> ACT HWDGE ring, remainder -> gpsimd SWDGE): a single
    FIFO ring serializes descriptor generation and starves engines.
  * The first two supertiles' input and last two supertiles' output
    move in 2048-column pieces to shorten the pipeline fill/drain.
"""

import os

import numpy as np

B = 1048576
N_CORES = 8
BC = B // N_CORES  # 131072 batch rows per core
NS = 32
NT = 48
K = NS + NT  # 80 contraction rows: states then shifted barriers

F_SUPER = 8192  # batch columns per DMA super-tile (16KB/partition fp16)
NSUP = BC // F_SUPER
F_PSUM = 2048  # batch columns per PSUM tile / ACT op (4 banks)
F_MM = 512  # batch columns per matmul (PE moving-dim max, one bank)

T = 298.15
K_B = 1.380649e-23
H = 6.62607015e-34
R = 0.008314462618
EYRING_PREFACTOR = K_B * T / H
RT = R * T
INV_RT = float(np.float32(1.0 / RT))  # reference casts 1/RT to f32
LN_PREF = float(np.log(EYRING_PREFACTOR))

_cached = {}


def _build_program(m_out, bias_val):
    from concourse import bacc, mybir
    from concourse.tile import TileContext

    nc = bacc.Bacc(
        None, target_bir_lowering=False, debug=False, num_devices=N_CORES
    )
    x = nc.dram_tensor(
        "x", [NSUP, K, F_SUPER], mybir.dt.float16, kind="ExternalInput"
    )
    w = nc.dram_tensor("w", [K, m_out], mybir.dt.float16, kind="ExternalInput")
    y = nc.dram_tensor(
        "y", [NSUP, m_out, F_SUPER], mybir.dt.bfloat16, kind="ExternalOutput"
    )

    exp = mybir.ActivationFunctionType.Exp

    with TileContext(nc) as tc:
        m16 = (m_out // 16) * 16
        with (
            tc.tile_pool(name="consts", bufs=1) as cpool,
            tc.tile_pool(name="inp", bufs=5) as ipool,
            tc.tile_pool(name="outp", bufs=5) as opool,
            tc.tile_pool(name="psum", bufs=2, space="PSUM") as ppool,
        ):
            wt = cpool.tile([K, m_out], mybir.dt.float16)
            nc.sync.dma_start(wt[:], w[:])
            bias_t = cpool.tile([128, 1], mybir.dt.float32)
            nc.vector.memset(bias_t[:], bias_val)

            for t in range(NSUP):
                xt = ipool.tile([K, F_SUPER], mybir.dt.float16, name="xt", tag="xt")
                if t <= 1:
                    # smaller first pieces so the PE starts ~8us earlier
                    for q in range(F_SUPER // F_PSUM):
                        qs = slice(q * F_PSUM, (q + 1) * F_PSUM)
                        nc.sync.dma_start(xt[:, qs], x[t][:, qs])
                else:
                    nc.sync.dma_start(xt[:], x[t])
                out = opool.tile(
                    [m_out, F_SUPER], mybir.dt.bfloat16, name="out", tag="out"
                )
                tail = t >= NSUP - 2  # drain the last tiles in small chunks
                for p in range(F_SUPER // F_PSUM):
                    ps = ppool.tile(
                        [m_out, F_PSUM], mybir.dt.float32, name="ps", tag="ps"
                    )
                    for m in range(F_PSUM // F_MM):
                        a = p * F_PSUM + m * F_MM
                        s = slice(m * F_MM, (m + 1) * F_MM)
                        nc.tensor.matmul(
                            ps[:, s], wt[:], xt[:, a : a + F_MM],
                            start=True, stop=True,
                        )
                    po = slice(p * F_PSUM, (p + 1) * F_PSUM)
                    nc.scalar.activation(
                        out[:, po], ps[:],
                        exp, bias=bias_t[:m_out], scale=INV_RT,
                    )
                    if tail and m16:
                        eng = nc.scalar if p % 2 == 0 else nc.sync
                        eng.dma_start(y[t, :m16, po], out[:m16, po])
                if not tail and m16:
                    nc.scalar.dma_start(y[t, :m16, :], out[:m16, :])
                if m16 < m_out:
                    nc.gpsimd.dma_start(y[t, m16:, :], out[m16:m_out, :])
    nc.compile()
    return nc


def _host_prep(state_energies, barrier_energies, from_idx, to_idx, reversible):
    se = np.asarray(state_energies, dtype=np.float32)
    be = np.asarray(barrier_energies, dtype=np.float32)
    fi = np.asarray(from_idx).astype(np.int64)
    ti = np.asarray(to_idx).astype(np.int64)
    rv = np.asarray(reversible).astype(bool)

    # Shift barriers by their (rounded) mean so fp16 keeps ~4 more
    # absolute bits; folded back exactly through the activation bias.
    c_shift = float(np.round(np.float64(be[:4096].mean())))

    x = np.empty((K, B), np.float16)
    x[0:NS] = se.T
    x[NS:] = (be - np.float32(c_shift)).T

    rev_idx = np.flatnonzero(rv)  # transitions with a reverse rate
    n_rev = len(rev_idx)
    m_out = NT + n_rev

    w = np.zeros((K, m_out), np.float16)
    cols = np.arange(NT)
    w[fi, cols] = 1.0
    w[NS + cols, cols] = -1.0
    if n_rev:
        rcols = NT + np.arange(n_rev)
        w[ti[rev_idx], rcols] = 1.0
        w[NS + rev_idx, rcols] = -1.0
    bias_val = LN_PREF - c_shift * INV_RT
    return x, w, rev_idx, m_out, bias_val


last_results = None


def kernel(state_energies, barrier_energies, from_idx, to_idx, reversible):
    global last_results
    from concourse.bass_utils import run_bass_kernel_spmd

    x, w, rev_idx, m_out, bias_val = _host_prep(
        state_energies, barrier_energies, from_idx, to_idx, reversible
    )

    key = (m_out, bias_val)
    if key not in _cached:
        _cached[key] = _build_program(m_out, bias_val)
    nc = _cached[key]

    in_maps = []
    for c in range(N_CORES):
        sl = slice(c * BC, (c + 1) * BC)
        # tile-major: (NSUP, K, F_SUPER), each supertile contiguous
        xc = np.ascontiguousarray(
            x[:, sl].reshape(K, NSUP, F_SUPER).swapaxes(0, 1)
        )
        in_maps.append({"x": xc, "w": w})

    res = run_bass_kernel_spmd(
        nc,
        in_maps,
        core_ids=list(range(N_CORES)),
        trace=bool(int(os.environ.get("KERNEL_TRACE", "0"))),
    )
    last_results = res

    n_rev = len(rev_idx)
    forward = np.empty((B, NT), np.float32)
    reverse = np.zeros((B, NT), np.float32)
    for c, r in enumerate(res.results):
        yc = np.asarray(r["y"]).reshape(NSUP, m_out, F_SUPER)
        # bf16 -> f32 via bit shift (exact, faster than astype)
        yf = (yc.view(np.uint16).astype(np.uint32) << 16).view(np.float32)
        cb = slice(c * BC, (c + 1) * BC)
        forward[cb] = yf[:, :NT, :].swapaxes(1, 2).reshape(BC, NT)
        if n_rev:
            reverse[cb][:, rev_idx] = yf[:, NT:, :].swapaxes(1, 2).reshape(
                BC, n_rev
            )
    return forward, reverse


# revision 29
# speedup vs baseline: 1.0133x; 1.0133x over previous
"""Trainium2 Bass kernel for nn_EnergyToRateConverter.

Computes Eyring rates  fwd = pref*exp(-(bar - G_from)/RT),
rev = reversible ? pref*exp(-(bar - G_to)/RT) : 0  for B=1M batch rows.

Strategy (pure data parallel over 8 cores, batch split 8 ways):
  * Host transposes inputs into one feature-major fp16 tensor
    X = [state.T; (barrier - C).T] of shape (80, B).  Subtracting the
    barrier mean C (~40) first puts barriers in the same fp16 binade as
    the state energies, so a single fp16 pass already hits ~1.3e-2
    worst-case relative error (gate is 2e-2) without a second
    residual-correction matmul pass.
  * One constant matmul W.T @ X per 512-column chunk fuses the
    per-transition gather AND the barrier subtraction:
        W[from_idx[j], j] = 1 (fwd cols) / W[to_idx[j], j] = 1 (rev)
        W[32+j, j] = -1  (subtract barrier j)
    Output rows are [48 fwd | n_rev rev] with no padding; rates for
    non-reversible transitions are never computed.
  * ScalarE evaluates out = exp(psum*inv_rt + (ln(pref) - C*inv_rt))
    straight from PSUM, writing bf16 (exponent range of f32, 2^-9
    rounding) — halving output DMA bytes vs f32.
  * DRAM tensors are tile-major: each 8192-column supertile is one
    contiguous 1.3/1.1 MB DRAM block (16KB per descriptor row), which
    keeps the SDMA engines in long sequential HBM bursts.
  * Output DMAs are issued as a 64-row chunk + 4-row remainder: the
    HWDGE spreads a DMA over the largest divisor of its descriptor
    count <= 16, so 64 rows ride all 16 SDMA engines (68 would use 4).
  * Traffic is split over three DMA queues (input -> SP HWDGE ring,
    output -> ACT HWDGE ring, remainder -> gpsimd SWDGE): a single
    FIFO ring serializes descriptor generation and starves engines.
  * The first two supertiles' input and last two supertiles' output
    move in 2048-column pieces to shorten the pipeline fill/drain.
"""

import os

import numpy as np

B = 1048576
N_CORES = 8
BC = B // N_CORES  # 131072 batch rows per core
NS = 32
NT = 48
K = NS + NT  # 80 contraction rows: states then shifted barriers

F_SUPER = 8192  # batch columns per DMA super-tile (16KB/partition fp16)
NSUP = BC // F_SUPER
F_PSUM = 2048  # batch columns per PSUM tile / ACT op (4 banks)
F_MM = 512  # batch columns per matmul (PE moving-dim max, one bank)

T = 298.15
K_B = 1.380649e-23
H = 6.62607015e-34
R = 0.008314462618
EYRING_PREFACTOR = K_B * T / H
RT = R * T
INV_RT = float(np.float32(1.0 / RT))  # reference casts 1/RT to f32
LN_PREF = float(np.log(EYRING_PREFACTOR))

_cached = {}


def _build_program(m_out, bias_val):
    from concourse import bacc, mybir
    from concourse.tile import TileContext

    nc = bacc.Bacc(
        None, target_bir_lowering=False, debug=False, num_devices=N_CORES
    )
    x = nc.dram_tensor(
        "x", [NSUP, K, F_SUPER], mybir.dt.float16, kind="ExternalInput"
    )
    w = nc.dram_tensor("w", [K, m_out], mybir.dt.float16, kind="ExternalInput")
    y = nc.dram_tensor(
        "y", [NSUP, m_out, F_SUPER], mybir.dt.bfloat16, kind="ExternalOutput"
    )

    exp = mybir.ActivationFunctionType.Exp

    with TileContext(nc) as tc:
        m16 = (m_out // 16) * 16
        with (
            tc.tile_pool(name="consts", bufs=1) as cpool,
            tc.tile_pool(name="inp", bufs=5) as ipool,
            tc.tile_pool(name="outp", bufs=5) as opool,
            tc.tile_pool(name="psum", bufs=2, space="PSUM") as ppool,
        ):
            wt = cpool.tile([K, m_out], mybir.dt.float16)
            nc.gpsimd.dma_start(wt[:], w[:])  # keep the SP ring clear
            bias_t = cpool.tile([128, 1], mybir.dt.float32)
            nc.vector.memset(bias_t[:], bias_val)

            for t in range(NSUP):
                xt = ipool.tile([K, F_SUPER], mybir.dt.float16, name="xt", tag="xt")
                if t <= 1:
                    # smaller first pieces so the PE starts ~8us earlier
                    for q in range(F_SUPER // F_PSUM):
                        qs = slice(q * F_PSUM, (q + 1) * F_PSUM)
                        nc.sync.dma_start(xt[:, qs], x[t][:, qs])
                else:
                    nc.sync.dma_start(xt[:], x[t])
                out = opool.tile(
                    [m_out, F_SUPER], mybir.dt.bfloat16, name="out", tag="out"
                )
                tail = t >= NSUP - 2  # drain the last tiles in small chunks
                for p in range(F_SUPER // F_PSUM):
                    ps = ppool.tile(
                        [m_out, F_PSUM], mybir.dt.float32, name="ps", tag="ps"
                    )
                    for m in range(F_PSUM // F_MM):
                        a = p * F_PSUM + m * F_MM
                        s = slice(m * F_MM, (m + 1) * F_MM)
                        nc.tensor.matmul(
                            ps[:, s], wt[:], xt[:, a : a + F_MM],
                            start=True, stop=True,
                        )
                    po = slice(p * F_PSUM, (p + 1) * F_PSUM)
                    nc.scalar.activation(
                        out[:, po], ps[:],
                        exp, bias=bias_t[:m_out], scale=INV_RT,
                    )
                    if tail and m16:
                        eng = nc.scalar if p % 2 == 0 else nc.sync
                        eng.dma_start(y[t, :m16, po], out[:m16, po])
                        if m16 < m_out:
                            eng2 = nc.sync if p % 2 == 0 else nc.scalar
                            eng2.dma_start(y[t, m16:, po], out[m16:m_out, po])
                if not tail and m16:
                    nc.scalar.dma_start(y[t, :m16, :], out[:m16, :])
                if not tail and m16 < m_out:
                    nc.gpsimd.dma_start(y[t, m16:, :], out[m16:m_out, :])
    nc.compile()
    return nc


def _host_prep(state_energies, barrier_energies, from_idx, to_idx, reversible):
    se = np.asarray(state_energies, dtype=np.float32)
    be = np.asarray(barrier_energies, dtype=np.float32)
    fi = np.asarray(from_idx).astype(np.int64)
    ti = np.asarray(to_idx).astype(np.int64)
    rv = np.asarray(reversible).astype(bool)

    # Shift barriers by their (rounded) mean so fp16 keeps ~4 more
    # absolute bits; folded back exactly through the activation bias.
    c_shift = float(np.round(np.float64(be[:4096].mean())))

    x = np.empty((K, B), np.float16)
    x[0:NS] = se.T
    x[NS:] = (be - np.float32(c_shift)).T

    rev_idx = np.flatnonzero(rv)  # transitions with a reverse rate
    n_rev = len(rev_idx)
    m_out = NT + n_rev

    w = np.zeros((K, m_out), np.float16)
    cols = np.arange(NT)
    w[fi, cols] = 1.0
    w[NS + cols, cols] = -1.0
    if n_rev:
        rcols = NT + np.arange(n_rev)
        w[ti[rev_idx], rcols] = 1.0
        w[NS + rev_idx, rcols] = -1.0
    bias_val = LN_PREF - c_shift * INV_RT
    return x, w, rev_idx, m_out, bias_val


last_results = None


def kernel(state_energies, barrier_energies, from_idx, to_idx, reversible):
    global last_results
    from concourse.bass_utils import run_bass_kernel_spmd

    x, w, rev_idx, m_out, bias_val = _host_prep(
        state_energies, barrier_energies, from_idx, to_idx, reversible
    )

    key = (m_out, bias_val)
    if key not in _cached:
        _cached[key] = _build_program(m_out, bias_val)
    nc = _cached[key]

    in_maps = []
    for c in range(N_CORES):
        sl = slice(c * BC, (c + 1) * BC)
        # tile-major: (NSUP, K, F_SUPER), each supertile contiguous
        xc = np.ascontiguousarray(
            x[:, sl].reshape(K, NSUP, F_SUPER).swapaxes(0, 1)
        )
        in_maps.append({"x": xc, "w": w})

    res = run_bass_kernel_spmd(
        nc,
        in_maps,
        core_ids=list(range(N_CORES)),
        trace=bool(int(os.environ.get("KERNEL_TRACE", "0"))),
    )
    last_results = res

    n_rev = len(rev_idx)
    forward = np.empty((B, NT), np.float32)
    reverse = np.zeros((B, NT), np.float32)
    for c, r in enumerate(res.results):
        yc = np.asarray(r["y"]).reshape(NSUP, m_out, F_SUPER)
        # bf16 -> f32 via bit shift (exact, faster than astype)
        yf = (yc.view(np.uint16).astype(np.uint32) << 16).view(np.float32)
        cb = slice(c * BC, (c + 1) * BC)
        forward[cb] = yf[:, :NT, :].swapaxes(1, 2).reshape(BC, NT)
        if n_rev:
            reverse[cb][:, rev_idx] = yf[:, NT:, :].swapaxes(1, 2).reshape(
                BC, n_rev
            )
    return forward, reverse
